# revision 74
# baseline (speedup 1.0000x reference)
"""Trainium2 Bass kernel for nn_Discriminator (dense MLP + pairwise L1 diversity).

SPMD over 8 cores, data-parallel over the N=1024 rows (P=128 rows/core).
Dense layers run in bf16 (fp32 PSUM accumulate). The diversity term

    div[j,k] = sum_i exp( - sum_d |M[i,k,d] - M[j,k,d]| ),  M = h @ Wd + bd

uses |B - s| = 2*relu(B - s) - B + s per (k,d):
  - DVE tensor_scalar(subtract, max) in bf16 4x mode produces A_d tiles
    (one d per unit offloaded to ACT Relu where the span is wide);
  - PE identity matmuls (bf16, 1 cy/row) accumulate the A_d over d into
    PSUM, one pair pre-added on DVE to balance engines;
  - a K=1 ones-row matmul adds the per-column -Sb/2 row (bf16, 1 cy/row);
  - ACT activation(Exp, scale=-2, bias=-Ss, accum_out=...) fuses the
    exponential with the row-sum over i.

Work split: core c handles kernel c for all eight 128-row J-blocks (units
0..7) plus kernels 8/9 for its own block (units 8, 9). Since the own
kernel's full 1024x1024 l1 matrix is local and symmetric, unit u computes
only columns [128u, 1024) (diag block included); the missing i < 128u
contributions are PE column-sums of earlier units' exp tiles, accumulated
in a PSUM row and transposed back into the div columns (~44% less
relu/stream/exp work for units 0..7).

Collectives: an AllToAll of 11-row kernel shards (a -Sb/2 row at
partition 0 + 10 M^T rows, computed on the owner via one block-diagonal
-0.5-ones matmul) hands each core its own kernel's rows; an AllGather of
rows 88..109 supplies kernels 8/9; a second AllToAll returns div columns
to their row owners, fired right after unit 7 so it overlaps units 8/9.

Per-unit scalars come from one 11-row PE transpose per J-block into a
stride-12 PSUM tile (negsb at even offsets; odd-offset bf16 PSUM reads
fail the hardware ISA check) drained by two full-tile copies; nss is read
from the same bf16 -Sb/2 values the ones-rows use, so exp(0)=1 is exact.
B tiles are per-d HWDGE(sync)/SWDGE(pool) DMA row-broadcasts issued in
consumption order (d0 straight from the scattered recv blocks); kernels
8/9 ride two mega-broadcasts behind them. The ACT queue carries only
activations (DMAs on it stall the exps behind their issue latency), and
ACT stays on the single exp/relu/copy table set: LN's rstd is a DVE-only
fast inverse sqrt (bit-trick seed + 2 Newton steps), and LeakyReLU is a
fused scalar_tensor_tensor. M travels in bf16; PSUM/LN stay fp32
(rel err ~5e-3).
"""

import os
import sys

import numpy as np

sys.path.insert(0, "/opt/trn_rl_repo")

import concourse.bass as bass
import concourse.bacc as bacc
import concourse.tile as tile
from concourse import mybir
from concourse.bass_utils import run_bass_kernel_spmd

try:
    import ml_dtypes

    BF16_NP = ml_dtypes.bfloat16
except ImportError:  # pragma: no cover
    BF16_NP = None

F32 = mybir.dt.float32
BF16 = mybir.dt.bfloat16

N = 1024
NF = 512
HID = 256
NK = 10
KD = 10
MB = NK * KD  # 100
CAT = HID + NK  # 266
EPS = 1e-3
ALPHA = 0.3
NCORES = 8
P = N // NCORES  # 128 rows per core
ROWS11 = 11  # 10 M^T dims + 1 negsb row per kernel shard
MROWS = ROWS11 * NK  # 110

AF = mybir.ActivationFunctionType
ALU = mybir.AluOpType

# d-slices pre-added pairwise on DVE before the PE streams (per unit)
COMBINES = 1
POOL_RELU = False
ACT_W = 640
N_WARM = 24


def _chunks(total, size):
    out = []
    o = 0
    while o < total:
        out.append((o, min(size, total - o)))
        o += size
    return out


def build_program(stage="full"):
    nc = bacc.Bacc(
        "TRN2",
        target_bir_lowering=False,
        debug=False,
        num_devices=NCORES,
    )

    # ---- per-core external inputs ----
    xT = nc.dram_tensor("xT", [NF, P], BF16, kind="ExternalInput")
    W0 = nc.dram_tensor("W0", [NF, HID], BF16, kind="ExternalInput")
    b0c = nc.dram_tensor("b0c", [HID, 1], F32, kind="ExternalInput")
    Wd0 = nc.dram_tensor("Wd0", [HID, MB], BF16, kind="ExternalInput")
    bd0c = nc.dram_tensor("bd0c", [MB, 1], F32, kind="ExternalInput")
    beta0b = nc.dram_tensor("beta0b", [P, CAT], F32, kind="ExternalInput")
    W1 = nc.dram_tensor("W1", [CAT, HID], BF16, kind="ExternalInput")
    b1c = nc.dram_tensor("b1c", [HID, 1], F32, kind="ExternalInput")
    Wd1 = nc.dram_tensor("Wd1", [HID, MB], BF16, kind="ExternalInput")
    bd1c = nc.dram_tensor("bd1c", [MB, 1], F32, kind="ExternalInput")
    beta1b = nc.dram_tensor("beta1b", [P, CAT], F32, kind="ExternalInput")
    Wfb = nc.dram_tensor("Wfb", [P, CAT], F32, kind="ExternalInput")
    bfc = nc.dram_tensor("bfc", [P, 1], F32, kind="ExternalInput")
    y_out = nc.dram_tensor("y", [P, 1], F32, kind="ExternalOutput")

    # ---- NEFF-embedded constants ----
    ident_f32 = nc.inline_tensor(np.eye(128, dtype=np.float32), name="ident_f32")
    ident_bf16 = nc.inline_tensor(
        np.eye(128).astype(BF16_NP), name="ident_bf16"
    )
    ones1_bf16 = nc.inline_tensor(
        np.ones((1, 128)).astype(BF16_NP), name="ones1_bf16"
    )
    onesc_bf16 = nc.inline_tensor(
        np.ones((128, 1)).astype(BF16_NP), name="onesc_bf16"
    )
    _kblk = np.zeros((MB, NK))
    for _k in range(NK):
        _kblk[_k * KD:(_k + 1) * KD, _k] = -0.5
    kblk_bf16 = nc.inline_tensor(_kblk.astype(BF16_NP), name="kblk_bf16")

    with tile.TileContext(nc, num_cores=NCORES) as tc:
        dram = tc.alloc_tile_pool(name="dram", bufs=1, space="DRAM")
        m_loc = [dram.tile([MROWS, P], BF16, name=f"m_loc{b}") for b in range(2)]
        m_gath = [
            dram.tile(
                [NCORES, 2 * ROWS11, P], BF16,
                addr_space=("Local" if stage == "nocc" else "Shared"),
                name=f"m_gath{b}",
            )
            for b in range(2)
        ]
        # AllToAll of kernel shards 0..7 (11 rows each): every core receives
        # its own kernel's rows (incl. the negsb row) from all peers
        mtam_recv = [
            dram.tile([NCORES, ROWS11, P], BF16, name=f"mtam_r{b}")
            for b in range(2)
        ]
        # own kernel rows + kernels 8/9 rows assembled contiguously in DRAM
        # (broadcast DMAs need a contiguous DRAM source row)
        mtA_dram = [dram.tile([KD, N], BF16, name=f"mtA_d{b}") for b in range(2)]
        mt8_dram = [dram.tile([KD, N], BF16, name=f"mt8_d{b}") for b in range(2)]
        mt9_dram = [dram.tile([KD, N], BF16, name=f"mt9_d{b}") for b in range(2)]
        a2a_send = [dram.tile([NCORES, P], F32, name=f"a2a_s{b}") for b in range(2)]
        a2a_recv = [
            dram.tile([NCORES, P], F32, name=f"a2a_r{b}") for b in range(2)
        ]
        consts = tc.alloc_tile_pool(name="consts", bufs=1)
        acts = tc.alloc_tile_pool(name="acts", bufs=1)
        mtiles = tc.alloc_tile_pool(name="mtiles", bufs=2)
        bpool = tc.alloc_tile_pool(name="bpool", bufs=2)
        apool = tc.alloc_tile_pool(name="apool", bufs=10)
        cpool = tc.alloc_tile_pool(name="cpool", bufs=2)
        epool = tc.alloc_tile_pool(name="epool", bufs=10)
        small = tc.alloc_tile_pool(name="small", bufs=2)
        ps_small = tc.alloc_tile_pool(name="ps_small", bufs=1, space="PSUM")
        ps_prep = tc.alloc_tile_pool(name="ps_prep", bufs=1, space="PSUM")
        ps_col = tc.alloc_tile_pool(name="ps_col", bufs=1, space="PSUM")
        ps_l1 = tc.alloc_tile_pool(name="ps_l1", bufs=2, space="PSUM")

        def ap_of(t, ap, extra_off=0):
            return bass.AP(tensor=t.tensor, offset=t.offset + extra_off, ap=ap)

        # ---------- load constants ----------
        # startup-critical consts via HWDGE (sync), each k-chunked weight
        # merged into a single [128, n*cols] tile via one strided DMA;
        # late-needed block-1/LN/head weights ride the Pool queue
        def load(dram_t, shape, dtype=F32, name=None, late=False, src_ap=None):
            t = consts.tile(shape, dtype, name=name)
            # late consts ride the Pool queue, but are emitted only after
            # block 0's collectives (emit_late_consts) so they don't block
            # the m-chain; ones1/selc ride the scalar queue ahead of its
            # first real work
            q = nc.gpsimd if late else nc.sync
            q.dma_start(out=t, in_=(src_ap if src_ap is not None else dram_t))
            return t

        def load_chunked(dram_t, rows, cols, nch, dtype=BF16, name=None,
                         late=False):
            # [nch*128, cols] dram -> [128, nch*cols] sbuf, one DMA
            t = load(
                dram_t, [128, nch * cols], dtype, name=name, late=late,
                src_ap=ap_of(dram_t[:, :],
                             [[cols, 128], [128 * cols, nch], [1, cols]]),
            )
            return [t[:, k * cols : (k + 1) * cols] for k in range(nch)]

        xT_sb = load_chunked(xT, NF, P, 4, name="xTall")
        w0_sb = load_chunked(W0, NF, HID, 4, name="w0all")
        # small startup-critical scalars ride the (empty-early) scalar
        # queue so the dense bias / M bias / negsb matmul aren't gated by
        # the sync queue's DMA issue cadence
        b0_t = consts.tile([128, 2], F32, name="b0all")
        nc.scalar.dma_start(out=b0_t,
                            in_=ap_of(b0c[:, :], [[1, 128], [128, 2]]))
        b0_sb = [b0_t[:, 0:1], b0_t[:, 1:2]]
        bd0_sb = consts.tile([MB, 1], F32, name="bd0")
        nc.scalar.dma_start(out=bd0_sb, in_=bd0c[:, :])
        kblk = consts.tile([MB, NK], BF16, name="kblk")
        nc.scalar.dma_start(out=kblk, in_=kblk_bf16[:, :])
        wd0_sb = load_chunked(Wd0, HID, MB, 2, name="wd0all")
        idb = load(ident_bf16[:, :], [128, 128], BF16, name="idb")
        idf = load(ident_f32[:, :], [128, 128], name="idf")
        ones1 = consts.tile([1, 128], BF16, name="ones1")
        nc.scalar.dma_start(out=ones1, in_=ones1_bf16[:, :])
        onesc = consts.tile([128, 1], BF16, name="onesc")
        nc.scalar.dma_start(out=onesc, in_=onesc_bf16[:, :])

        late_sb = {}

        def emit_late_consts():
            w1_sb = load_chunked(W1, 256, HID, 2, name="w1all", late=True)
            w1c3 = load(W1[256:CAT, :], [10, HID], BF16, name="w1c3",
                        late=True)
            late_sb["w1"] = w1_sb + [w1c3[:, :]]
            late_sb["wd1"] = load_chunked(Wd1, HID, MB, 2, name="wd1all",
                                          late=True)
            b1_t = load(b1c[:, :], [128, 2], name="b1all", late=True,
                        src_ap=ap_of(b1c[:, :], [[1, 128], [128, 2]]))
            late_sb["b1"] = [b1_t[:, 0:1], b1_t[:, 1:2]]
            late_sb["bd1"] = load(bd1c[:, :], [MB, 1], name="bd1", late=True)
            late_sb["beta"] = [
                load(beta0b[:, :], [P, CAT], name="beta0", late=True),
                load(beta1b[:, :], [P, CAT], name="beta1", late=True),
            ]
            late_sb["wf"] = load(Wfb[:, :], [P, CAT], name="wf", late=True)
            late_sb["bf"] = load(bfc[:, :], [P, 1], name="bf", late=True)

        # ---------- one block ----------
        def block(b, prevT, w_sb, b_sb, wd_sb, bd_sb, after_ship=None):
            """prevT: list of (tile, psize) feature-major bf16 chunks.

            Returns [P, CAT] fp32 tile = LeakyReLU(LN(concat(h, div)) + beta).
            """
            # h^T = W^T @ prev + b   (feature-major, HID x P as 2 chunks)
            hT = []
            for mi, (mo, msz) in enumerate(_chunks(HID, 128)):
                ps = ps_small.tile([128, P], F32, tag="ps_small")
                for ki, (wt, (pt, psz)) in enumerate(zip(w_sb, prevT)):
                    nc.tensor.matmul(
                        ps[:msz, :],
                        wt[:psz, mo : mo + msz],
                        pt,
                        start=(ki == 0),
                        stop=(ki == len(w_sb) - 1),
                    )
                ht = acts.tile([msz, P], BF16, name=f"hT{b}_{mi}")
                nc.vector.tensor_scalar(
                    out=ht, in0=ps[:msz, :], scalar1=b_sb[mi], scalar2=None,
                    op0=ALU.add,
                )
                hT.append((ht, msz))

            # M^T = Wd^T @ h + bd   [100, 128] bf16
            ps_m = ps_small.tile([MB, P], F32, tag="ps_small")
            for ki, ((ht, _), wdt) in enumerate(zip(hT, wd_sb)):
                nc.tensor.matmul(
                    ps_m,
                    wdt,
                    ht,
                    start=(ki == 0),
                    stop=(ki == len(wd_sb) - 1),
                )
            mT = mtiles.tile([MB, P], BF16, tag="mT")
            nc.vector.tensor_scalar(
                out=mT, in0=ps_m, scalar1=bd_sb, scalar2=None, op0=ALU.add
            )

            # negsb rows: -Sb/2 per kernel for own rows (shipped with M^T);
            # one block-diagonal ones matmul straight off mT keeps this on
            # the m_loc critical path w/o waiting for the m_row transpose
            ps_nb = ps_small.tile([NK, P], F32, tag="ps_small")
            nc.tensor.matmul(ps_nb, kblk, mT, start=True, stop=True)
            negsbT = mtiles.tile([NK, P], BF16, tag="negsbT")
            nc.vector.tensor_copy(negsbT, ps_nb)

            # own M rows (row-major fp32) for units 8/9 scalars
            ps_t = ps_small.tile([128, MB], BF16, tag="ps_small")
            nc.tensor.transpose(ps_t[:, :], mT, idb[:MB, :MB])
            m_row = mtiles.tile([P, MB], F32, tag="m_row")
            nc.vector.tensor_copy(m_row, ps_t[:, :MB])

            # keep the PE pipeline warm through the collective wait: the
            # pstate model drops to cold after any idle, which would tax the
            # first units' streams; these dummies are never read
            if N_WARM:
                ps_w = ps_l1.tile([P, N], F32, tag="psl")
                for _ in range(N_WARM):
                    nc.tensor.matmul(
                        ps_w[:, 0:128], idb, idb,
                        start=True, stop=True, skip_group_check=True,
                    )

            # ---- ship M^T + negsb rows (kernel-sharded layout, negsb
            # first in each 11-row shard so receivers get it at partition 0)
            nc.sync.dma_start(
                out=ap_of(m_loc[b][:, :], [[ROWS11 * P, NK], [P, KD], [1, P]],
                          extra_off=P),
                in_=mT,
            )
            nc.sync.dma_start(
                out=ap_of(m_loc[b][:, :], [[ROWS11 * P, NK], [1, P]]),
                in_=negsbT,
            )

            if stage == "nocc":
                nc.gpsimd.dma_start(
                    out=mtam_recv[b][:, :, :],
                    in_=m_loc[b][0 : NCORES * ROWS11, :],
                )
            else:
                nc.gpsimd.collective_compute(
                    "AllToAll",
                    ALU.bypass,
                    replica_groups=[list(range(NCORES))],
                    ins=[m_loc[b][0 : NCORES * ROWS11, :]],
                    outs=[mtam_recv[b][:, :, :]],
                )

            def emit_gather():
                if stage == "nocc":
                    src89 = m_loc[b][NCORES * ROWS11 : MROWS, :]
                    nc.gpsimd.dma_start(
                        out=m_gath[b][:, :, :],
                        in_=ap_of(src89,
                                  [[0, NCORES], [P, 2 * ROWS11], [1, P]]),
                    )
                else:
                    nc.gpsimd.collective_compute(
                        "AllGather",
                        ALU.bypass,
                        replica_groups=[list(range(NCORES))],
                        ins=[m_loc[b][NCORES * ROWS11 : MROWS, :]],
                        outs=[m_gath[b][:, :, :]],
                    )

            # ---- assemble SBUF views + contiguous DRAM broadcast sources ----
            recv = mtam_recv[b][:, :, :]
            gath = m_gath[b][:, :, :]
            # own kernel: one [11, N] tile = negsb row 0 + 10 M rows
            mtA11 = mtiles.tile([ROWS11, N], BF16, tag="mtA11")
            nc.gpsimd.dma_start(
                out=mtA11,
                in_=ap_of(recv, [[P, ROWS11], [ROWS11 * P, NCORES], [1, P]]),
            )

            # ---- broadcast B tiles: HWDGE(sync)/SWDGE(pool) row-broadcasts,
            # per-d slices in consumption order; d=0 broadcasts straight from
            # the scattered recv blocks (skips the mtA_dram hop + its sem)
            btA = bpool.tile([P, KD * N], BF16, tag="btA")
            nc.sync.dma_start(
                out=mtA_dram[b][:, :],
                in_=ap_of(recv, [[P, KD], [ROWS11 * P, NCORES], [1, P]],
                          extra_off=P),
            )
            nc.sync.dma_start(
                out=btA[:, 0:N],
                in_=ap_of(recv, [[0, P], [ROWS11 * P, NCORES], [1, P]],
                          extra_off=P),
            )
            for d in range(1, KD):
                src = mtA_dram[b][d : d + 1, :]
                (nc.sync if d % 2 == 0 else nc.gpsimd).dma_start(
                    out=btA[:, d * N : (d + 1) * N],
                    in_=ap_of(src, [[0, P], [1, N]]),
                )
            negsbA = mtA11[0:1, :]
            emit_gather()
            # kernels 8/9: DRAM broadcast sources (M rows only) + one
            # [11, N] SBUF tile each (negsb at partition 0 + 10 M rows)
            negsb89 = []
            for k, mtk_dram in ((0, mt8_dram), (1, mt9_dram)):
                nc.gpsimd.dma_start(
                    out=mtk_dram[b][:, :],
                    in_=ap_of(gath,
                              [[P, KD], [2 * ROWS11 * P, NCORES], [1, P]],
                              extra_off=(k * ROWS11 + 1) * P),
                )
                t = mtiles.tile([1, N], BF16, tag=f"nsb8{k}")
                nc.gpsimd.dma_start(
                    out=t,
                    in_=ap_of(gath, [[2 * ROWS11 * P, NCORES], [1, P]],
                              extra_off=k * ROWS11 * P),
                )
                negsb89.append(t)
            bt89 = []
            for k, mtk_dram in ((0, mt8_dram), (1, mt9_dram)):
                bt = bpool.tile([P, KD * N], BF16, tag=f"bt8{k}")
                s = mtk_dram[b][0:1, :]
                nc.gpsimd.dma_start(out=bt, in_=ap_of(s, [[0, P], [1, KD * N]]))
                bt89.append(bt)
            if after_ship is not None:
                after_ship()

            # ---- per-unit scalars, one fused PSUM tile ----
            # one 11-row transpose per unit yields the negsb slice (col 0,
            # the nss source) AND the 10 scal values (cols 1..10) at once;
            # cols 88..97 hold negsbT transposed (own rows, all kernels)
            # stride-12 unit layout keeps each bf16 transpose output at an
            # even column (PSUM matmul writes must be 4-byte aligned)
            ps_pp = ps_prep.tile([128, 108], BF16, tag="ps_prep")
            for u in range(NCORES):
                nc.tensor.transpose(
                    ps_pp[:, u * 12 : u * 12 + ROWS11],
                    mtA11[:, u * P : (u + 1) * P],
                    idb[:ROWS11, :ROWS11],
                )
            nc.tensor.transpose(
                ps_pp[:, 96:106], negsbT, idb[:KD, :KD]
            )
            # full-tile aligned PSUM->SBUF copies (odd bf16 PSUM offsets
            # fail the ISA check); per-unit scalars are then SBUF slices
            ppAll = small.tile([P, 108], F32, tag="ppAll")
            nc.vector.tensor_copy(ppAll, ps_pp)
            nppAll = small.tile([P, 108], F32, tag="nppAll")
            nc.vector.tensor_scalar(
                out=nppAll, in0=ps_pp,
                scalar1=-1.0, scalar2=None, op0=ALU.mult,
            )
            nscal89 = small.tile([P, 2 * KD], F32, tag="nscal89")
            nc.vector.tensor_scalar(
                out=nscal89, in0=m_row[:, NCORES * KD : MB],
                scalar1=-1.0, scalar2=None, op0=ALU.mult,
            )
            nssA = small.tile([P, NCORES], F32, tag="nssA")
            nc.vector.tensor_scalar(
                out=nssA,
                in0=bass.AP(tensor=ppAll.tensor, offset=ppAll.offset,
                            ap=[ppAll.ap[0], [12, NCORES]]),
                scalar1=2.0, scalar2=None, op0=ALU.mult,
            )
            # units 8/9: nss = -Ss(own rows) = 2 * (-Sb/2) from negsbT
            nss89 = small.tile([P, 2], F32, tag="nss89")
            nc.vector.tensor_scalar(
                out=nss89, in0=ppAll[:, 104:106],
                scalar1=2.0, scalar2=None, op0=ALU.mult,
            )

            cat = acts.tile([P, CAT], F32, name=f"cat{b}")
            divsend = acts.tile([P, NCORES], F32, name=f"divsend{b}")

            # h rows into cat[:, :256] via PE transposes of hT (bf16);
            # emitted early — fills PE/DVE during the m-chain wait
            for mi, (ht, msz) in enumerate(hT):
                ps_t2 = ps_small.tile([128, P], BF16, tag="ps_small")
                nc.tensor.transpose(ps_t2[:, :msz], ht, idb[:msz, :msz])
                nc.vector.tensor_copy(
                    cat[:, mi * 128 : mi * 128 + msz], ps_t2[:, :msz]
                )

            # ---- unit loop ----
            # Symmetry: for the own kernel the full 1024x1024 l1 matrix
            # lives on this core, so unit u computes only columns
            # [128u, 1024) (diag block included). The missing i < 128u
            # contributions are column sums of earlier units' exp tiles,
            # accumulated into a PSUM row and transposed into divsend.
            def span_chunks(off):
                if off < 512:
                    return [(off, 512 - off), (512, 512)]
                return [(off, N - off)]

            escr_tiles = {}

            def unit_ctx(u):
                ctx = {}
                if u < NCORES:
                    ctx["off"] = u * P
                    ctx["bt"] = btA
                    ctx["negsb"] = negsbA
                    ctx["scal"] = lambda d, _u=u: ppAll[
                        :, _u * 12 + 1 + d : _u * 12 + 2 + d]
                    ctx["nscal"] = lambda d, _u=u: nppAll[
                        :, _u * 12 + 1 + d : _u * 12 + 2 + d]
                    ctx["accum"] = divsend[:, u : u + 1]
                    ctx["nss"] = nssA[:, u : u + 1]
                else:
                    k = u - NCORES
                    ctx["off"] = 0
                    ctx["bt"] = bt89[k]
                    ctx["negsb"] = negsb89[k]
                    ctx["scal"] = lambda d, _k=k: m_row[
                        :, (NCORES + _k) * KD + d : (NCORES + _k) * KD + d + 1]
                    ctx["nscal"] = lambda d, _k=k: nscal89[
                        :, _k * KD + d : _k * KD + d + 1]
                    ctx["accum"] = cat[:, HID + u : HID + u + 1]
                    ctx["nss"] = nss89[:, k : k + 1]
                ctx["u"] = u
                ctx["w"] = N - ctx["off"]
                ctx["psl"] = ps_l1.tile([P, N], F32, tag="psl", name=f"psl_{u}")
                return ctx

            def relu_d(c, d, on_act=False):
                off = c["off"]
                at = apool.tile([P, N], BF16, tag="at", name=f"at{d}")
                if on_act:
                    # ACT Relu(x - s) with the negated per-partition bias
                    nc.scalar.activation(
                        at[:, off:N], c["bt"][:, d * N + off : (d + 1) * N],
                        AF.Relu, bias=c["nscal"](d), scale=1.0,
                    )
                else:
                    nc.vector.tensor_scalar(
                        out=at[:, off:N],
                        in0=c["bt"][:, d * N + off : (d + 1) * N],
                        scalar1=c["scal"](d),
                        scalar2=0.0,
                        op0=ALU.subtract,
                        op1=ALU.max,
                    )
                return at

            def stream(c, at, first):
                for ho, hsz in span_chunks(c["off"]):
                    nc.tensor.matmul(
                        c["psl"][:, ho : ho + hsz],
                        idb,
                        at[:, ho : ho + hsz],
                        start=first,
                        stop=False,
                    )

            def unit_tail(c):
                off, u = c["off"], c["u"]
                n_direct = KD - 2 * COMBINES
                for ci in range(COMBINES):
                    lo = n_direct + 2 * ci
                    a0, a1 = relu_d(c, lo), relu_d(c, lo + 1)
                    comb = cpool.tile([P, N], BF16, tag="comb")
                    nc.vector.tensor_add(
                        comb[:, off:N], a0[:, off:N], a1[:, off:N]
                    )
                    stream(c, comb, False)
                for ho, hsz in span_chunks(off):
                    nc.tensor.matmul(
                        c["psl"][:, ho : ho + hsz],
                        ones1,
                        c["negsb"][:, ho : ho + hsz],
                        start=False,
                        stop=True,
                    )
                escr = epool.tile([P, N], BF16, tag="escr")
                nc.scalar.activation(
                    escr[:, off:N], c["psl"][:, off:N], AF.Exp, bias=c["nss"],
                    scale=-2.0, accum_out=c["accum"],
                )
                escr_tiles[u] = escr

            def emit_units(ulist):
                # relu/stream emission interleaved across the group per d, so
                # the DVE queue never head-of-line blocks on a late broadcast
                # slice and each slice feeds every unit of the group
                ctxs = [unit_ctx(u) for u in ulist]
                n_direct = KD - 2 * COMBINES
                for d in range(n_direct):
                    for c in ctxs:
                        on_act = c["w"] >= ACT_W and d == n_direct - 1
                        stream(c, relu_d(c, d, on_act=on_act), d == 0)
                for c in ctxs:
                    unit_tail(c)

            pscol = ps_col.tile([1, N], F32, tag="pscol")

            def emit_colsum(u):
                # column sums of unit u's exp tile over its off-diag columns
                # [128(u+1), 1024); accumulated into the pscol row
                cs_off = (u + 1) * P
                escr = escr_tiles[u]
                for co, csz in span_chunks(cs_off):
                    lastu = 2 if co < 512 else NCORES - 2
                    nc.tensor.matmul(
                        pscol[0:1, co : co + csz],
                        onesc,
                        escr[:, co : co + csz],
                        start=(u == 0),
                        stop=(u == lastu),
                        skip_group_check=True,
                    )

            for u in range(NCORES):
                emit_units([u])
            for u in range(NCORES - 1):
                emit_colsum(u)
            # fold the transposed colsum pieces into divsend cols 1..7
            cs_sb = mtiles.tile([1, N], BF16, tag="cs_sb")
            nc.vector.tensor_copy(cs_sb[0:1, P:N], pscol[0:1, P:N])
            ps_ct = ps_prep.tile([128, 108], BF16, tag="ps_prep")
            for v in range(1, NCORES):
                nc.tensor.transpose(
                    ps_ct[:, 2 * (v - 1) : 2 * (v - 1) + 1],
                    cs_sb[0:1, v * P : (v + 1) * P],
                    idb[:1, :1],
                )
            nc.vector.tensor_tensor(
                out=divsend[:, 1:NCORES],
                in0=divsend[:, 1:NCORES],
                in1=bass.AP(tensor=ps_ct.tensor, offset=ps_ct.offset,
                            ap=[ps_ct.ap[0], [2, NCORES - 1]]),
                op=ALU.add,
            )

            # ---- exchange div columns (AllToAll): the send side fires as
            # soon as unit 7's exp lands, overlapping units 8/9 compute ----
            ps_ds = ps_small.tile([128, P], F32, tag="ps_small")
            nc.tensor.transpose(ps_ds[:NCORES, :], divsend, idf)
            dsend_sb = small.tile([NCORES, P], F32, tag="dsend")
            nc.vector.tensor_copy(dsend_sb, ps_ds[:NCORES, :])
            nc.sync.dma_start(out=a2a_send[b][:, :], in_=dsend_sb)
            if stage == "nocc":
                nc.gpsimd.dma_start(
                    out=a2a_recv[b][:, :], in_=a2a_send[b][:, :]
                )
            else:
                nc.gpsimd.collective_compute(
                    "AllToAll",
                    ALU.bypass,
                    replica_groups=[list(range(NCORES))],
                    ins=[a2a_send[b][:, :]],
                    outs=[a2a_recv[b][:, :]],
                )
            drecv_sb = small.tile([NCORES, P], F32, tag="drecv")
            nc.gpsimd.dma_start(out=drecv_sb, in_=a2a_recv[b][:, :])

            emit_units([NCORES])
            emit_units([NCORES + 1])

            ps_dr = ps_small.tile([128, NCORES], F32, tag="ps_small")
            nc.tensor.transpose(
                ps_dr[:, :NCORES], drecv_sb, idf[:NCORES, :NCORES]
            )
            nc.vector.tensor_copy(
                cat[:, HID : HID + NCORES], ps_dr[:, :NCORES]
            )

            # LayerNorm (center+scale, beta only). bn_stats in two groups:
            # the 256 h columns are ready mid-unit-loop, only the 10 div
            # columns land late
            stats = small.tile([P, 12], F32, tag="stats")
            nc.vector.bn_stats(out=stats[:, 0:6], in_=cat[:, :HID])
            nc.vector.bn_stats(out=stats[:, 6:12], in_=cat[:, HID:CAT])
            mv = small.tile([P, 2], F32, tag="mv")
            nc.vector.bn_aggr(out=mv, in_=stats)
            # rstd = 1/sqrt(var + eps) entirely on DVE via the bit-trick
            # seed + two Newton steps (keeps ACT on the Exp table; hardware
            # DVE has no sqrt/pow)
            ve = small.tile([P, 1], F32, tag="ve")
            nc.vector.tensor_scalar(
                out=ve, in0=mv[:, 1:2], scalar1=EPS, scalar2=None, op0=ALU.add
            )
            yi = small.tile([P, 1], F32, tag="rstd_y")
            nc.vector.tensor_scalar(
                out=yi.bitcast(mybir.dt.int32), in0=ve.bitcast(mybir.dt.int32),
                scalar1=1, scalar2=None, op0=ALU.arith_shift_right,
            )
            y0 = small.tile([P, 1], F32, tag="rstd_y0")
            nc.vector.tensor_scalar(
                out=y0.bitcast(mybir.dt.int32), in0=yi.bitcast(mybir.dt.int32),
                scalar1=-1, scalar2=0x5F3759DF, op0=ALU.mult, op1=ALU.add,
            )
            rstd = y0
            for _ in range(2):
                t2 = small.tile([P, 1], F32, tag="rstd_t2")
                nc.vector.tensor_tensor(out=t2, in0=rstd, in1=rstd, op=ALU.mult)
                nc.vector.tensor_tensor(out=t2, in0=t2, in1=ve, op=ALU.mult)
                nc.vector.tensor_scalar(
                    out=t2, in0=t2, scalar1=-0.5, scalar2=1.5,
                    op0=ALU.mult, op1=ALU.add,
                )
                ystep = small.tile([P, 1], F32, tag="rstd_ys")
                nc.vector.tensor_tensor(out=ystep, in0=rstd, in1=t2, op=ALU.mult)
                rstd = ystep
            catn = acts.tile([P, CAT], F32, name=f"catn{b}")
            nc.vector.tensor_scalar(
                out=catn,
                in0=cat,
                scalar1=mv[:, 0:1],
                scalar2=rstd,
                op0=ALU.subtract,
                op1=ALU.mult,
            )
            nc.vector.tensor_add(catn, catn, late_sb["beta"][b])
            # leaky relu in one fused op: max(0.3*x, x)
            hout = acts.tile([P, CAT], F32, name=f"hout{b}")
            nc.vector.scalar_tensor_tensor(
                out=hout, in0=catn, scalar=ALPHA, in1=catn,
                op0=ALU.mult, op1=ALU.max,
            )
            return hout

        # ---------- block 0 ----------
        prev0 = [(t, 128) for t in xT_sb]
        h1 = block(0, prev0, w0_sb, b0_sb, wd0_sb, bd0_sb,
                   after_ship=emit_late_consts)

        # transpose h1 -> feature-major bf16 chunks for block 1; all three
        # transposes land in one PSUM tile so a single copy drains them
        ps_t = ps_small.tile([128, 3 * P], F32, tag="ps_small")
        for ci, (co, csz) in enumerate(_chunks(CAT, 128)):
            nc.tensor.transpose(
                ps_t[:csz, ci * P : (ci + 1) * P], h1[:, co : co + csz], idf
            )
        h1T_all = acts.tile([128, 3 * P], BF16, name="h1T_all")
        nc.vector.tensor_copy(h1T_all, ps_t)
        h1T = [
            (h1T_all[:csz, ci * P : (ci + 1) * P], csz)
            for ci, (co, csz) in enumerate(_chunks(CAT, 128))
        ]

        # ---------- block 1 ----------
        h2 = block(1, h1T, late_sb["w1"], late_sb["b1"], late_sb["wd1"],
                   late_sb["bd1"])

        # ---------- critic head: y = h2 @ Wf + bf ----------
        hw = acts.tile([P, CAT], F32, name="hw")
        yacc = small.tile([P, 1], F32, tag="yacc")
        nc.vector.tensor_mul(hw, h2, late_sb["wf"])
        nc.vector.tensor_reduce(
            out=yacc, in_=hw, axis=mybir.AxisListType.X, op=ALU.add
        )
        ysb = small.tile([P, 1], F32, tag="ysb")
        nc.vector.tensor_scalar(
            out=ysb, in0=yacc, scalar1=late_sb["bf"], scalar2=None, op0=ALU.add
        )
        nc.sync.dma_start(out=y_out[:, :], in_=ysb)

        ps_l1.release()
        ps_col.release()
        ps_prep.release()
        ps_small.release()
        small.release()
        epool.release()
        cpool.release()
        apool.release()
        bpool.release()
        mtiles.release()
        acts.release()
        consts.release()
        dram.release()

    nc.compile()
    return nc


_NC_CACHE = {}


def _get_nc():
    stage = os.environ.get("KERNEL_STAGE", "full")
    if stage not in _NC_CACHE:
        _NC_CACHE[stage] = build_program(stage)
    return _NC_CACHE[stage]


def _make_in_maps(inputs):
    if BF16_NP is None:
        raise RuntimeError("ml_dtypes required for bf16 inputs")
    f = lambda a: np.ascontiguousarray(np.asarray(a, dtype=np.float32))
    bf = lambda a: np.ascontiguousarray(np.asarray(a, dtype=np.float32)).astype(
        BF16_NP
    )
    x = f(inputs["x"])
    shared = {
        "W0": bf(inputs["W0"]),
        "b0c": f(inputs["b0"]).reshape(HID, 1),
        "Wd0": bf(inputs["Wd0"]),
        "bd0c": f(inputs["bd0"]).reshape(MB, 1),
        "beta0b": np.ascontiguousarray(
            np.broadcast_to(f(inputs["beta0"]), (P, CAT))
        ),
        "W1": bf(inputs["W1"]),
        "b1c": f(inputs["b1"]).reshape(HID, 1),
        "Wd1": bf(inputs["Wd1"]),
        "bd1c": f(inputs["bd1"]).reshape(MB, 1),
        "beta1b": np.ascontiguousarray(
            np.broadcast_to(f(inputs["beta1"]), (P, CAT))
        ),
        "Wfb": np.ascontiguousarray(
            np.broadcast_to(f(inputs["Wf"]).reshape(1, CAT), (P, CAT))
        ),
        "bfc": np.full((P, 1), float(np.asarray(inputs["bf"]).reshape(-1)[0]),
                       dtype=np.float32),
    }
    in_maps = []
    for c in range(NCORES):
        m = dict(shared)
        m["xT"] = np.ascontiguousarray(x[c * P : (c + 1) * P, :].T).astype(
            BF16_NP
        )
        in_maps.append(m)
    return in_maps


def run(inputs, **kw):
    nc = _get_nc()
    in_maps = _make_in_maps(inputs)
    res = run_bass_kernel_spmd(nc, in_maps, list(range(NCORES)), **kw)
    y = np.concatenate([res.results[c]["y"] for c in range(NCORES)], axis=0)
    return y.astype(np.float32), res


def kernel(**inputs) -> np.ndarray:
    y, _ = run(inputs)
    return y


# revision 75
# speedup vs baseline: 1.0051x; 1.0051x over previous
"""Trainium2 Bass kernel for nn_Discriminator (dense MLP + pairwise L1 diversity).

SPMD over 8 cores, data-parallel over the N=1024 rows (P=128 rows/core).
Dense layers run in bf16 (fp32 PSUM accumulate). The diversity term

    div[j,k] = sum_i exp( - sum_d |M[i,k,d] - M[j,k,d]| ),  M = h @ Wd + bd

uses |B - s| = 2*relu(B - s) - B + s per (k,d):
  - DVE tensor_scalar(subtract, max) in bf16 4x mode produces A_d tiles
    (one d per unit offloaded to ACT Relu where the span is wide);
  - PE identity matmuls (bf16, 1 cy/row) accumulate the A_d over d into
    PSUM, one pair pre-added on DVE to balance engines;
  - a K=1 ones-row matmul adds the per-column -Sb/2 row (bf16, 1 cy/row);
  - ACT activation(Exp, scale=-2, bias=-Ss, accum_out=...) fuses the
    exponential with the row-sum over i.

Work split: core c handles kernel c for all eight 128-row J-blocks (units
0..7) plus kernels 8/9 for its own block (units 8, 9). Since the own
kernel's full 1024x1024 l1 matrix is local and symmetric, unit u computes
only columns [128u, 1024) (diag block included); the missing i < 128u
contributions are PE column-sums of earlier units' exp tiles, accumulated
in a PSUM row and transposed back into the div columns (~44% less
relu/stream/exp work for units 0..7).

Collectives: an AllToAll of 11-row kernel shards (a -Sb/2 row at
partition 0 + 10 M^T rows, computed on the owner via one block-diagonal
-0.5-ones matmul) hands each core its own kernel's rows; an AllGather of
rows 88..109 supplies kernels 8/9; a second AllToAll returns div columns
to their row owners, fired right after unit 7 so it overlaps units 8/9.

Per-unit scalars come from one 11-row PE transpose per J-block into a
stride-12 PSUM tile (negsb at even offsets; odd-offset bf16 PSUM reads
fail the hardware ISA check) drained by two full-tile copies; nss is read
from the same bf16 -Sb/2 values the ones-rows use, so exp(0)=1 is exact.
B tiles are per-d HWDGE(sync)/SWDGE(pool) DMA row-broadcasts issued in
consumption order (d0 straight from the scattered recv blocks); kernels
8/9 ride two mega-broadcasts behind them. The ACT queue carries only
activations (DMAs on it stall the exps behind their issue latency), and
ACT stays on the single exp/relu/copy table set: LN's rstd is a DVE-only
fast inverse sqrt (bit-trick seed + 2 Newton steps), and LeakyReLU is a
fused scalar_tensor_tensor. M travels in bf16; PSUM/LN stay fp32
(rel err ~5e-3).
"""

import os
import sys

import numpy as np

sys.path.insert(0, "/opt/trn_rl_repo")

import concourse.bass as bass
import concourse.bacc as bacc
import concourse.tile as tile
from concourse import mybir
from concourse.bass_utils import run_bass_kernel_spmd

try:
    import ml_dtypes

    BF16_NP = ml_dtypes.bfloat16
except ImportError:  # pragma: no cover
    BF16_NP = None

F32 = mybir.dt.float32
BF16 = mybir.dt.bfloat16

N = 1024
NF = 512
HID = 256
NK = 10
KD = 10
MB = NK * KD  # 100
CAT = HID + NK  # 266
EPS = 1e-3
ALPHA = 0.3
NCORES = 8
P = N // NCORES  # 128 rows per core
ROWS11 = 11  # 10 M^T dims + 1 negsb row per kernel shard
MROWS = ROWS11 * NK  # 110

AF = mybir.ActivationFunctionType
ALU = mybir.AluOpType

# d-slices pre-added pairwise on DVE before the PE streams (per unit)
COMBINES = 1
POOL_RELU = False
ACT_W = 640
N_WARM = 24


def _chunks(total, size):
    out = []
    o = 0
    while o < total:
        out.append((o, min(size, total - o)))
        o += size
    return out


def build_program(stage="full"):
    nc = bacc.Bacc(
        "TRN2",
        target_bir_lowering=False,
        debug=False,
        num_devices=NCORES,
    )

    # ---- per-core external inputs ----
    xT = nc.dram_tensor("xT", [NF, P], BF16, kind="ExternalInput")
    W0 = nc.dram_tensor("W0", [NF, HID], BF16, kind="ExternalInput")
    b0c = nc.dram_tensor("b0c", [HID, 1], F32, kind="ExternalInput")
    Wd0 = nc.dram_tensor("Wd0", [HID, MB], BF16, kind="ExternalInput")
    bd0c = nc.dram_tensor("bd0c", [MB, 1], F32, kind="ExternalInput")
    beta0b = nc.dram_tensor("beta0b", [P, CAT], F32, kind="ExternalInput")
    W1 = nc.dram_tensor("W1", [CAT, HID], BF16, kind="ExternalInput")
    b1c = nc.dram_tensor("b1c", [HID, 1], F32, kind="ExternalInput")
    Wd1 = nc.dram_tensor("Wd1", [HID, MB], BF16, kind="ExternalInput")
    bd1c = nc.dram_tensor("bd1c", [MB, 1], F32, kind="ExternalInput")
    beta1b = nc.dram_tensor("beta1b", [P, CAT], F32, kind="ExternalInput")
    Wfb = nc.dram_tensor("Wfb", [P, CAT], F32, kind="ExternalInput")
    bfc = nc.dram_tensor("bfc", [P, 1], F32, kind="ExternalInput")
    y_out = nc.dram_tensor("y", [P, 1], F32, kind="ExternalOutput")

    # ---- NEFF-embedded constants ----
    ident_f32 = nc.inline_tensor(np.eye(128, dtype=np.float32), name="ident_f32")
    ident_bf16 = nc.inline_tensor(
        np.eye(128).astype(BF16_NP), name="ident_bf16"
    )
    ones1_bf16 = nc.inline_tensor(
        np.ones((1, 128)).astype(BF16_NP), name="ones1_bf16"
    )
    onesc_bf16 = nc.inline_tensor(
        np.ones((128, 1)).astype(BF16_NP), name="onesc_bf16"
    )
    _kblk = np.zeros((MB, NK))
    for _k in range(NK):
        _kblk[_k * KD:(_k + 1) * KD, _k] = -0.5
    kblk_bf16 = nc.inline_tensor(_kblk.astype(BF16_NP), name="kblk_bf16")

    with tile.TileContext(nc, num_cores=NCORES) as tc:
        dram = tc.alloc_tile_pool(name="dram", bufs=1, space="DRAM")
        m_loc = [dram.tile([MROWS, P], BF16, name=f"m_loc{b}") for b in range(2)]
        m_gath = [
            dram.tile(
                [NCORES, 2 * ROWS11, P], BF16,
                addr_space=("Local" if stage == "nocc" else "Shared"),
                name=f"m_gath{b}",
            )
            for b in range(2)
        ]
        # AllToAll of kernel shards 0..7 (11 rows each): every core receives
        # its own kernel's rows (incl. the negsb row) from all peers
        mtam_recv = [
            dram.tile([NCORES, ROWS11, P], BF16, name=f"mtam_r{b}")
            for b in range(2)
        ]
        # own kernel rows + kernels 8/9 rows assembled contiguously in DRAM
        # (broadcast DMAs need a contiguous DRAM source row)
        mtA_dram = [dram.tile([KD, N], BF16, name=f"mtA_d{b}") for b in range(2)]
        mt8_dram = [dram.tile([KD, N], BF16, name=f"mt8_d{b}") for b in range(2)]
        mt9_dram = [dram.tile([KD, N], BF16, name=f"mt9_d{b}") for b in range(2)]
        a2a_send = [dram.tile([NCORES, P], F32, name=f"a2a_s{b}") for b in range(2)]
        a2a_recv = [
            dram.tile([NCORES, P], F32, name=f"a2a_r{b}") for b in range(2)
        ]
        consts = tc.alloc_tile_pool(name="consts", bufs=1)
        acts = tc.alloc_tile_pool(name="acts", bufs=1)
        mtiles = tc.alloc_tile_pool(name="mtiles", bufs=2)
        bpool = tc.alloc_tile_pool(name="bpool", bufs=2)
        apool = tc.alloc_tile_pool(name="apool", bufs=10)
        cpool = tc.alloc_tile_pool(name="cpool", bufs=2)
        epool = tc.alloc_tile_pool(name="epool", bufs=10)
        small = tc.alloc_tile_pool(name="small", bufs=2)
        ps_small = tc.alloc_tile_pool(name="ps_small", bufs=1, space="PSUM")
        ps_prep = tc.alloc_tile_pool(name="ps_prep", bufs=1, space="PSUM")
        ps_col = tc.alloc_tile_pool(name="ps_col", bufs=1, space="PSUM")
        ps_l1 = tc.alloc_tile_pool(name="ps_l1", bufs=2, space="PSUM")

        def ap_of(t, ap, extra_off=0):
            return bass.AP(tensor=t.tensor, offset=t.offset + extra_off, ap=ap)

        # ---------- load constants ----------
        # startup-critical consts via HWDGE (sync), each k-chunked weight
        # merged into a single [128, n*cols] tile via one strided DMA;
        # late-needed block-1/LN/head weights ride the Pool queue
        def load(dram_t, shape, dtype=F32, name=None, late=False, src_ap=None):
            t = consts.tile(shape, dtype, name=name)
            # late consts ride the Pool queue, but are emitted only after
            # block 0's collectives (emit_late_consts) so they don't block
            # the m-chain; ones1/selc ride the scalar queue ahead of its
            # first real work
            q = nc.gpsimd if late else nc.sync
            q.dma_start(out=t, in_=(src_ap if src_ap is not None else dram_t))
            return t

        def load_chunked(dram_t, rows, cols, nch, dtype=BF16, name=None,
                         late=False):
            # [nch*128, cols] dram -> [128, nch*cols] sbuf, one DMA
            t = load(
                dram_t, [128, nch * cols], dtype, name=name, late=late,
                src_ap=ap_of(dram_t[:, :],
                             [[cols, 128], [128 * cols, nch], [1, cols]]),
            )
            return [t[:, k * cols : (k + 1) * cols] for k in range(nch)]

        xT_sb = load_chunked(xT, NF, P, 4, name="xTall")
        w0_sb = load_chunked(W0, NF, HID, 4, name="w0all")
        b0_t = load(b0c[:, :], [128, 2], name="b0all",
                    src_ap=ap_of(b0c[:, :], [[1, 128], [128, 2]]))
        b0_sb = [b0_t[:, 0:1], b0_t[:, 1:2]]
        wd0_sb = load_chunked(Wd0, HID, MB, 2, name="wd0all")
        bd0_sb = load(bd0c[:, :], [MB, 1], name="bd0")
        kblk = load(kblk_bf16[:, :], [MB, NK], BF16, name="kblk")
        idb = load(ident_bf16[:, :], [128, 128], BF16, name="idb")
        idf = load(ident_f32[:, :], [128, 128], name="idf")
        ones1 = consts.tile([1, 128], BF16, name="ones1")
        nc.scalar.dma_start(out=ones1, in_=ones1_bf16[:, :])
        onesc = consts.tile([128, 1], BF16, name="onesc")
        nc.scalar.dma_start(out=onesc, in_=onesc_bf16[:, :])

        late_sb = {}

        def emit_late_consts():
            w1_sb = load_chunked(W1, 256, HID, 2, name="w1all", late=True)
            w1c3 = load(W1[256:CAT, :], [10, HID], BF16, name="w1c3",
                        late=True)
            late_sb["w1"] = w1_sb + [w1c3[:, :]]
            late_sb["wd1"] = load_chunked(Wd1, HID, MB, 2, name="wd1all",
                                          late=True)
            b1_t = load(b1c[:, :], [128, 2], name="b1all", late=True,
                        src_ap=ap_of(b1c[:, :], [[1, 128], [128, 2]]))
            late_sb["b1"] = [b1_t[:, 0:1], b1_t[:, 1:2]]
            late_sb["bd1"] = load(bd1c[:, :], [MB, 1], name="bd1", late=True)
            late_sb["beta"] = [
                load(beta0b[:, :], [P, CAT], name="beta0", late=True),
                load(beta1b[:, :], [P, CAT], name="beta1", late=True),
            ]
            late_sb["wf"] = load(Wfb[:, :], [P, CAT], name="wf", late=True)
            late_sb["bf"] = load(bfc[:, :], [P, 1], name="bf", late=True)

        # ---------- one block ----------
        def block(b, prevT, w_sb, b_sb, wd_sb, bd_sb, after_ship=None):
            """prevT: list of (tile, psize) feature-major bf16 chunks.

            Returns [P, CAT] fp32 tile = LeakyReLU(LN(concat(h, div)) + beta).
            """
            # h^T = W^T @ prev + b   (feature-major, HID x P as 2 chunks)
            hT = []
            for mi, (mo, msz) in enumerate(_chunks(HID, 128)):
                ps = ps_small.tile([128, P], F32, tag="ps_small")
                for ki, (wt, (pt, psz)) in enumerate(zip(w_sb, prevT)):
                    nc.tensor.matmul(
                        ps[:msz, :],
                        wt[:psz, mo : mo + msz],
                        pt,
                        start=(ki == 0),
                        stop=(ki == len(w_sb) - 1),
                    )
                ht = acts.tile([msz, P], BF16, name=f"hT{b}_{mi}")
                nc.vector.tensor_scalar(
                    out=ht, in0=ps[:msz, :], scalar1=b_sb[mi], scalar2=None,
                    op0=ALU.add,
                )
                hT.append((ht, msz))

            # M^T = Wd^T @ h + bd   [100, 128] bf16
            ps_m = ps_small.tile([MB, P], F32, tag="ps_small")
            for ki, ((ht, _), wdt) in enumerate(zip(hT, wd_sb)):
                nc.tensor.matmul(
                    ps_m,
                    wdt,
                    ht,
                    start=(ki == 0),
                    stop=(ki == len(wd_sb) - 1),
                )
            mT = mtiles.tile([MB, P], BF16, tag="mT")
            nc.vector.tensor_scalar(
                out=mT, in0=ps_m, scalar1=bd_sb, scalar2=None, op0=ALU.add
            )

            # negsb rows: -Sb/2 per kernel for own rows (shipped with M^T);
            # one block-diagonal ones matmul straight off mT keeps this on
            # the m_loc critical path w/o waiting for the m_row transpose
            ps_nb = ps_small.tile([NK, P], F32, tag="ps_small")
            nc.tensor.matmul(ps_nb, kblk, mT, start=True, stop=True)
            negsbT = mtiles.tile([NK, P], BF16, tag="negsbT")
            nc.vector.tensor_copy(negsbT, ps_nb)

            # own M rows (row-major fp32) for units 8/9 scalars
            ps_t = ps_small.tile([128, MB], BF16, tag="ps_small")
            nc.tensor.transpose(ps_t[:, :], mT, idb[:MB, :MB])
            m_row = mtiles.tile([P, MB], F32, tag="m_row")
            nc.vector.tensor_copy(m_row, ps_t[:, :MB])

            # keep the PE pipeline warm through the collective wait: the
            # pstate model drops to cold after any idle, which would tax the
            # first units' streams; these dummies are never read
            if N_WARM:
                ps_w = ps_l1.tile([P, N], F32, tag="psl")
                for _ in range(N_WARM):
                    nc.tensor.matmul(
                        ps_w[:, 0:128], idb, idb,
                        start=True, stop=True, skip_group_check=True,
                    )

            # ---- ship M^T + negsb rows (kernel-sharded layout, negsb
            # first in each 11-row shard so receivers get it at partition 0)
            nc.sync.dma_start(
                out=ap_of(m_loc[b][:, :], [[ROWS11 * P, NK], [P, KD], [1, P]],
                          extra_off=P),
                in_=mT,
            )
            nc.sync.dma_start(
                out=ap_of(m_loc[b][:, :], [[ROWS11 * P, NK], [1, P]]),
                in_=negsbT,
            )

            if stage == "nocc":
                nc.gpsimd.dma_start(
                    out=mtam_recv[b][:, :, :],
                    in_=m_loc[b][0 : NCORES * ROWS11, :],
                )
            else:
                nc.gpsimd.collective_compute(
                    "AllToAll",
                    ALU.bypass,
                    replica_groups=[list(range(NCORES))],
                    ins=[m_loc[b][0 : NCORES * ROWS11, :]],
                    outs=[mtam_recv[b][:, :, :]],
                )

            def emit_gather():
                if stage == "nocc":
                    src89 = m_loc[b][NCORES * ROWS11 : MROWS, :]
                    nc.gpsimd.dma_start(
                        out=m_gath[b][:, :, :],
                        in_=ap_of(src89,
                                  [[0, NCORES], [P, 2 * ROWS11], [1, P]]),
                    )
                else:
                    nc.gpsimd.collective_compute(
                        "AllGather",
                        ALU.bypass,
                        replica_groups=[list(range(NCORES))],
                        ins=[m_loc[b][NCORES * ROWS11 : MROWS, :]],
                        outs=[m_gath[b][:, :, :]],
                    )

            # ---- assemble SBUF views + contiguous DRAM broadcast sources ----
            recv = mtam_recv[b][:, :, :]
            gath = m_gath[b][:, :, :]
            # own kernel: one [11, N] tile = negsb row 0 + 10 M rows
            mtA11 = mtiles.tile([ROWS11, N], BF16, tag="mtA11")
            nc.gpsimd.dma_start(
                out=mtA11,
                in_=ap_of(recv, [[P, ROWS11], [ROWS11 * P, NCORES], [1, P]]),
            )

            # ---- broadcast B tiles: HWDGE(sync)/SWDGE(pool) row-broadcasts,
            # per-d slices in consumption order; d=0 broadcasts straight from
            # the scattered recv blocks (skips the mtA_dram hop + its sem)
            btA = bpool.tile([P, KD * N], BF16, tag="btA")
            nc.sync.dma_start(
                out=mtA_dram[b][:, :],
                in_=ap_of(recv, [[P, KD], [ROWS11 * P, NCORES], [1, P]],
                          extra_off=P),
            )
            nc.sync.dma_start(
                out=btA[:, 0:N],
                in_=ap_of(recv, [[0, P], [ROWS11 * P, NCORES], [1, P]],
                          extra_off=P),
            )
            for d in range(1, KD):
                src = mtA_dram[b][d : d + 1, :]
                (nc.sync if d % 2 == 0 else nc.gpsimd).dma_start(
                    out=btA[:, d * N : (d + 1) * N],
                    in_=ap_of(src, [[0, P], [1, N]]),
                )
            negsbA = mtA11[0:1, :]
            emit_gather()
            # kernels 8/9: DRAM broadcast sources (M rows only) + one
            # [11, N] SBUF tile each (negsb at partition 0 + 10 M rows)
            negsb89 = []
            for k, mtk_dram in ((0, mt8_dram), (1, mt9_dram)):
                nc.gpsimd.dma_start(
                    out=mtk_dram[b][:, :],
                    in_=ap_of(gath,
                              [[P, KD], [2 * ROWS11 * P, NCORES], [1, P]],
                              extra_off=(k * ROWS11 + 1) * P),
                )
                t = mtiles.tile([1, N], BF16, tag=f"nsb8{k}")
                nc.gpsimd.dma_start(
                    out=t,
                    in_=ap_of(gath, [[2 * ROWS11 * P, NCORES], [1, P]],
                              extra_off=k * ROWS11 * P),
                )
                negsb89.append(t)
            bt89 = []
            for k, mtk_dram in ((0, mt8_dram), (1, mt9_dram)):
                bt = bpool.tile([P, KD * N], BF16, tag=f"bt8{k}")
                s = mtk_dram[b][0:1, :]
                nc.gpsimd.dma_start(out=bt, in_=ap_of(s, [[0, P], [1, KD * N]]))
                bt89.append(bt)
            if after_ship is not None:
                after_ship()

            # ---- per-unit scalars, one fused PSUM tile ----
            # one 11-row transpose per unit yields the negsb slice (col 0,
            # the nss source) AND the 10 scal values (cols 1..10) at once;
            # cols 88..97 hold negsbT transposed (own rows, all kernels)
            # stride-12 unit layout keeps each bf16 transpose output at an
            # even column (PSUM matmul writes must be 4-byte aligned)
            ps_pp = ps_prep.tile([128, 108], BF16, tag="ps_prep")
            for u in range(NCORES):
                nc.tensor.transpose(
                    ps_pp[:, u * 12 : u * 12 + ROWS11],
                    mtA11[:, u * P : (u + 1) * P],
                    idb[:ROWS11, :ROWS11],
                )
            nc.tensor.transpose(
                ps_pp[:, 96:106], negsbT, idb[:KD, :KD]
            )
            # full-tile aligned PSUM->SBUF copies (odd bf16 PSUM offsets
            # fail the ISA check); per-unit scalars are then SBUF slices
            ppAll = small.tile([P, 108], F32, tag="ppAll")
            nc.vector.tensor_copy(ppAll, ps_pp)
            nppAll = small.tile([P, 108], F32, tag="nppAll")
            nc.vector.tensor_scalar(
                out=nppAll, in0=ps_pp,
                scalar1=-1.0, scalar2=None, op0=ALU.mult,
            )
            nscal89 = small.tile([P, 2 * KD], F32, tag="nscal89")
            nc.vector.tensor_scalar(
                out=nscal89, in0=m_row[:, NCORES * KD : MB],
                scalar1=-1.0, scalar2=None, op0=ALU.mult,
            )
            nssA = small.tile([P, NCORES], F32, tag="nssA")
            nc.vector.tensor_scalar(
                out=nssA,
                in0=bass.AP(tensor=ppAll.tensor, offset=ppAll.offset,
                            ap=[ppAll.ap[0], [12, NCORES]]),
                scalar1=2.0, scalar2=None, op0=ALU.mult,
            )
            # units 8/9: nss = -Ss(own rows) = 2 * (-Sb/2) from negsbT
            nss89 = small.tile([P, 2], F32, tag="nss89")
            nc.vector.tensor_scalar(
                out=nss89, in0=ppAll[:, 104:106],
                scalar1=2.0, scalar2=None, op0=ALU.mult,
            )

            cat = acts.tile([P, CAT], F32, name=f"cat{b}")
            divsend = acts.tile([P, NCORES], F32, name=f"divsend{b}")

            # h rows into cat[:, :256] via PE transposes of hT (bf16);
            # emitted early — fills PE/DVE during the m-chain wait
            for mi, (ht, msz) in enumerate(hT):
                ps_t2 = ps_small.tile([128, P], BF16, tag="ps_small")
                nc.tensor.transpose(ps_t2[:, :msz], ht, idb[:msz, :msz])
                nc.vector.tensor_copy(
                    cat[:, mi * 128 : mi * 128 + msz], ps_t2[:, :msz]
                )

            # ---- unit loop ----
            # Symmetry: for the own kernel the full 1024x1024 l1 matrix
            # lives on this core, so unit u computes only columns
            # [128u, 1024) (diag block included). The missing i < 128u
            # contributions are column sums of earlier units' exp tiles,
            # accumulated into a PSUM row and transposed into divsend.
            def span_chunks(off):
                if off < 512:
                    return [(off, 512 - off), (512, 512)]
                return [(off, N - off)]

            escr_tiles = {}

            def unit_ctx(u):
                ctx = {}
                if u < NCORES:
                    ctx["off"] = u * P
                    ctx["bt"] = btA
                    ctx["negsb"] = negsbA
                    ctx["scal"] = lambda d, _u=u: ppAll[
                        :, _u * 12 + 1 + d : _u * 12 + 2 + d]
                    ctx["nscal"] = lambda d, _u=u: nppAll[
                        :, _u * 12 + 1 + d : _u * 12 + 2 + d]
                    ctx["accum"] = divsend[:, u : u + 1]
                    ctx["nss"] = nssA[:, u : u + 1]
                else:
                    k = u - NCORES
                    ctx["off"] = 0
                    ctx["bt"] = bt89[k]
                    ctx["negsb"] = negsb89[k]
                    ctx["scal"] = lambda d, _k=k: m_row[
                        :, (NCORES + _k) * KD + d : (NCORES + _k) * KD + d + 1]
                    ctx["nscal"] = lambda d, _k=k: nscal89[
                        :, _k * KD + d : _k * KD + d + 1]
                    ctx["accum"] = cat[:, HID + u : HID + u + 1]
                    ctx["nss"] = nss89[:, k : k + 1]
                ctx["u"] = u
                ctx["w"] = N - ctx["off"]
                ctx["psl"] = ps_l1.tile([P, N], F32, tag="psl", name=f"psl_{u}")
                return ctx

            def relu_d(c, d, on_act=False):
                off = c["off"]
                at = apool.tile([P, N], BF16, tag="at", name=f"at{d}")
                if on_act:
                    # ACT Relu(x - s) with the negated per-partition bias
                    nc.scalar.activation(
                        at[:, off:N], c["bt"][:, d * N + off : (d + 1) * N],
                        AF.Relu, bias=c["nscal"](d), scale=1.0,
                    )
                else:
                    nc.vector.tensor_scalar(
                        out=at[:, off:N],
                        in0=c["bt"][:, d * N + off : (d + 1) * N],
                        scalar1=c["scal"](d),
                        scalar2=0.0,
                        op0=ALU.subtract,
                        op1=ALU.max,
                    )
                return at

            def stream(c, at, first):
                for ho, hsz in span_chunks(c["off"]):
                    nc.tensor.matmul(
                        c["psl"][:, ho : ho + hsz],
                        idb,
                        at[:, ho : ho + hsz],
                        start=first,
                        stop=False,
                    )

            def unit_tail(c):
                off, u = c["off"], c["u"]
                n_direct = KD - 2 * COMBINES
                for ci in range(COMBINES):
                    lo = n_direct + 2 * ci
                    a0, a1 = relu_d(c, lo), relu_d(c, lo + 1)
                    comb = cpool.tile([P, N], BF16, tag="comb")
                    nc.vector.tensor_add(
                        comb[:, off:N], a0[:, off:N], a1[:, off:N]
                    )
                    stream(c, comb, False)
                for ho, hsz in span_chunks(off):
                    nc.tensor.matmul(
                        c["psl"][:, ho : ho + hsz],
                        ones1,
                        c["negsb"][:, ho : ho + hsz],
                        start=False,
                        stop=True,
                    )
                escr = epool.tile([P, N], BF16, tag="escr")
                nc.scalar.activation(
                    escr[:, off:N], c["psl"][:, off:N], AF.Exp, bias=c["nss"],
                    scale=-2.0, accum_out=c["accum"],
                )
                escr_tiles[u] = escr

            def emit_units(ulist):
                # relu/stream emission interleaved across the group per d, so
                # the DVE queue never head-of-line blocks on a late broadcast
                # slice and each slice feeds every unit of the group
                ctxs = [unit_ctx(u) for u in ulist]
                n_direct = KD - 2 * COMBINES
                for d in range(n_direct):
                    for c in ctxs:
                        on_act = c["w"] >= ACT_W and d == n_direct - 1
                        stream(c, relu_d(c, d, on_act=on_act), d == 0)
                for c in ctxs:
                    unit_tail(c)

            pscol = ps_col.tile([1, N], F32, tag="pscol")

            def emit_colsum(u):
                # column sums of unit u's exp tile over its off-diag columns
                # [128(u+1), 1024); accumulated into the pscol row
                cs_off = (u + 1) * P
                escr = escr_tiles[u]
                for co, csz in span_chunks(cs_off):
                    lastu = 2 if co < 512 else NCORES - 2
                    nc.tensor.matmul(
                        pscol[0:1, co : co + csz],
                        onesc,
                        escr[:, co : co + csz],
                        start=(u == 0),
                        stop=(u == lastu),
                        skip_group_check=True,
                    )

            for u in range(NCORES):
                emit_units([u])
            for u in range(NCORES - 1):
                emit_colsum(u)
            # fold the transposed colsum pieces into divsend cols 1..7
            cs_sb = mtiles.tile([1, N], BF16, tag="cs_sb")
            nc.vector.tensor_copy(cs_sb[0:1, P:N], pscol[0:1, P:N])
            ps_ct = ps_prep.tile([128, 108], BF16, tag="ps_prep")
            for v in range(1, NCORES):
                nc.tensor.transpose(
                    ps_ct[:, 2 * (v - 1) : 2 * (v - 1) + 1],
                    cs_sb[0:1, v * P : (v + 1) * P],
                    idb[:1, :1],
                )
            nc.vector.tensor_tensor(
                out=divsend[:, 1:NCORES],
                in0=divsend[:, 1:NCORES],
                in1=bass.AP(tensor=ps_ct.tensor, offset=ps_ct.offset,
                            ap=[ps_ct.ap[0], [2, NCORES - 1]]),
                op=ALU.add,
            )

            # ---- exchange div columns (AllToAll): the send side fires as
            # soon as unit 7's exp lands, overlapping units 8/9 compute ----
            ps_ds = ps_small.tile([128, P], F32, tag="ps_small")
            nc.tensor.transpose(ps_ds[:NCORES, :], divsend, idf)
            dsend_sb = small.tile([NCORES, P], F32, tag="dsend")
            nc.vector.tensor_copy(dsend_sb, ps_ds[:NCORES, :])
            nc.sync.dma_start(out=a2a_send[b][:, :], in_=dsend_sb)
            if stage == "nocc":
                nc.gpsimd.dma_start(
                    out=a2a_recv[b][:, :], in_=a2a_send[b][:, :]
                )
            else:
                nc.gpsimd.collective_compute(
                    "AllToAll",
                    ALU.bypass,
                    replica_groups=[list(range(NCORES))],
                    ins=[a2a_send[b][:, :]],
                    outs=[a2a_recv[b][:, :]],
                )
            drecv_sb = small.tile([NCORES, P], F32, tag="drecv")
            nc.gpsimd.dma_start(out=drecv_sb, in_=a2a_recv[b][:, :])

            emit_units([NCORES])
            emit_units([NCORES + 1])

            ps_dr = ps_small.tile([128, NCORES], F32, tag="ps_small")
            nc.tensor.transpose(
                ps_dr[:, :NCORES], drecv_sb, idf[:NCORES, :NCORES]
            )
            nc.vector.tensor_copy(
                cat[:, HID : HID + NCORES], ps_dr[:, :NCORES]
            )

            # LayerNorm (center+scale, beta only). bn_stats in two groups:
            # the 256 h columns are ready mid-unit-loop, only the 10 div
            # columns land late
            stats = small.tile([P, 12], F32, tag="stats")
            nc.vector.bn_stats(out=stats[:, 0:6], in_=cat[:, :HID])
            nc.vector.bn_stats(out=stats[:, 6:12], in_=cat[:, HID:CAT])
            mv = small.tile([P, 2], F32, tag="mv")
            nc.vector.bn_aggr(out=mv, in_=stats)
            # rstd = 1/sqrt(var + eps) entirely on DVE via the bit-trick
            # seed + two Newton steps (keeps ACT on the Exp table; hardware
            # DVE has no sqrt/pow)
            ve = small.tile([P, 1], F32, tag="ve")
            nc.vector.tensor_scalar(
                out=ve, in0=mv[:, 1:2], scalar1=EPS, scalar2=None, op0=ALU.add
            )
            yi = small.tile([P, 1], F32, tag="rstd_y")
            nc.vector.tensor_scalar(
                out=yi.bitcast(mybir.dt.int32), in0=ve.bitcast(mybir.dt.int32),
                scalar1=1, scalar2=None, op0=ALU.arith_shift_right,
            )
            y0 = small.tile([P, 1], F32, tag="rstd_y0")
            nc.vector.tensor_scalar(
                out=y0.bitcast(mybir.dt.int32), in0=yi.bitcast(mybir.dt.int32),
                scalar1=-1, scalar2=0x5F3759DF, op0=ALU.mult, op1=ALU.add,
            )
            rstd = y0
            for _ in range(2):
                t2 = small.tile([P, 1], F32, tag="rstd_t2")
                nc.vector.tensor_tensor(out=t2, in0=rstd, in1=rstd, op=ALU.mult)
                nc.vector.tensor_tensor(out=t2, in0=t2, in1=ve, op=ALU.mult)
                nc.vector.tensor_scalar(
                    out=t2, in0=t2, scalar1=-0.5, scalar2=1.5,
                    op0=ALU.mult, op1=ALU.add,
                )
                ystep = small.tile([P, 1], F32, tag="rstd_ys")
                nc.vector.tensor_tensor(out=ystep, in0=rstd, in1=t2, op=ALU.mult)
                rstd = ystep
            catn = acts.tile([P, CAT], F32, name=f"catn{b}")
            nc.vector.tensor_scalar(
                out=catn,
                in0=cat,
                scalar1=mv[:, 0:1],
                scalar2=rstd,
                op0=ALU.subtract,
                op1=ALU.mult,
            )
            nc.vector.tensor_add(catn, catn, late_sb["beta"][b])
            # leaky relu in one fused op: max(0.3*x, x)
            hout = acts.tile([P, CAT], F32, name=f"hout{b}")
            nc.vector.scalar_tensor_tensor(
                out=hout, in0=catn, scalar=ALPHA, in1=catn,
                op0=ALU.mult, op1=ALU.max,
            )
            return hout

        # ---------- block 0 ----------
        prev0 = [(t, 128) for t in xT_sb]
        h1 = block(0, prev0, w0_sb, b0_sb, wd0_sb, bd0_sb,
                   after_ship=emit_late_consts)

        # transpose h1 -> feature-major bf16 chunks for block 1; all three
        # transposes land in one PSUM tile so a single copy drains them
        ps_t = ps_small.tile([128, 3 * P], F32, tag="ps_small")
        for ci, (co, csz) in enumerate(_chunks(CAT, 128)):
            nc.tensor.transpose(
                ps_t[:csz, ci * P : (ci + 1) * P], h1[:, co : co + csz], idf
            )
        h1T_all = acts.tile([128, 3 * P], BF16, name="h1T_all")
        nc.vector.tensor_copy(h1T_all, ps_t)
        h1T = [
            (h1T_all[:csz, ci * P : (ci + 1) * P], csz)
            for ci, (co, csz) in enumerate(_chunks(CAT, 128))
        ]

        # ---------- block 1 ----------
        h2 = block(1, h1T, late_sb["w1"], late_sb["b1"], late_sb["wd1"],
                   late_sb["bd1"])

        # ---------- critic head: y = h2 @ Wf + bf ----------
        hw = acts.tile([P, CAT], F32, name="hw")
        yacc = small.tile([P, 1], F32, tag="yacc")
        nc.vector.tensor_mul(hw, h2, late_sb["wf"])
        nc.vector.tensor_reduce(
            out=yacc, in_=hw, axis=mybir.AxisListType.X, op=ALU.add
        )
        ysb = small.tile([P, 1], F32, tag="ysb")
        nc.vector.tensor_scalar(
            out=ysb, in0=yacc, scalar1=late_sb["bf"], scalar2=None, op0=ALU.add
        )
        nc.sync.dma_start(out=y_out[:, :], in_=ysb)

        ps_l1.release()
        ps_col.release()
        ps_prep.release()
        ps_small.release()
        small.release()
        epool.release()
        cpool.release()
        apool.release()
        bpool.release()
        mtiles.release()
        acts.release()
        consts.release()
        dram.release()

    nc.compile()
    return nc


_NC_CACHE = {}


def _get_nc():
    stage = os.environ.get("KERNEL_STAGE", "full")
    if stage not in _NC_CACHE:
        _NC_CACHE[stage] = build_program(stage)
    return _NC_CACHE[stage]


def _make_in_maps(inputs):
    if BF16_NP is None:
        raise RuntimeError("ml_dtypes required for bf16 inputs")
    f = lambda a: np.ascontiguousarray(np.asarray(a, dtype=np.float32))
    bf = lambda a: np.ascontiguousarray(np.asarray(a, dtype=np.float32)).astype(
        BF16_NP
    )
    x = f(inputs["x"])
    shared = {
        "W0": bf(inputs["W0"]),
        "b0c": f(inputs["b0"]).reshape(HID, 1),
        "Wd0": bf(inputs["Wd0"]),
        "bd0c": f(inputs["bd0"]).reshape(MB, 1),
        "beta0b": np.ascontiguousarray(
            np.broadcast_to(f(inputs["beta0"]), (P, CAT))
        ),
        "W1": bf(inputs["W1"]),
        "b1c": f(inputs["b1"]).reshape(HID, 1),
        "Wd1": bf(inputs["Wd1"]),
        "bd1c": f(inputs["bd1"]).reshape(MB, 1),
        "beta1b": np.ascontiguousarray(
            np.broadcast_to(f(inputs["beta1"]), (P, CAT))
        ),
        "Wfb": np.ascontiguousarray(
            np.broadcast_to(f(inputs["Wf"]).reshape(1, CAT), (P, CAT))
        ),
        "bfc": np.full((P, 1), float(np.asarray(inputs["bf"]).reshape(-1)[0]),
                       dtype=np.float32),
    }
    in_maps = []
    for c in range(NCORES):
        m = dict(shared)
        m["xT"] = np.ascontiguousarray(x[c * P : (c + 1) * P, :].T).astype(
            BF16_NP
        )
        in_maps.append(m)
    return in_maps


def run(inputs, **kw):
    nc = _get_nc()
    in_maps = _make_in_maps(inputs)
    res = run_bass_kernel_spmd(nc, in_maps, list(range(NCORES)), **kw)
    y = np.concatenate([res.results[c]["y"] for c in range(NCORES)], axis=0)
    return y.astype(np.float32), res


def kernel(**inputs) -> np.ndarray:
    y, _ = run(inputs)
    return y
